# revision 7
# baseline (speedup 1.0000x reference)
"""BiAttention (binary attention transformer block) Trainium2 kernel.

Forward-pass reduction of the reference:
  - softmax cancels:  stop_gradient(binq - soft) + soft == binq  (forward)
  - sign() is invariant to the positive per-row qkv weight scale
So per batch element (one per NeuronCore, 8 cores data-parallel):
  bq,bk,bv = sign(x @ sign(Wqkv).T)   split into heads
  A        = (bq @ bk.T > 0)          in {0,1}
  oo       = A @ bv                   exact small integers
  out      = (oo @ sign(Wproj).T) * mean(|Wproj|,axis=1) + b_proj
"""

import numpy as np

import concourse.bacc as bacc
import concourse.bass as bass
import concourse.mybir as mybir
import concourse.tile as tile
from concourse.masks import make_identity

FP32 = mybir.dt.float32
FP16 = mybir.dt.float16
FP8 = mybir.dt.float8e4
AF = mybir.ActivationFunctionType
ALU = mybir.AluOpType
DR = mybir.MatmulPerfMode.DoubleRow

B, N, C = 8, 1024, 768
H, D = 12, 64
C3 = 3 * C  # 2304
NK = C // 128  # 6 contraction chunks
NM = N // 128  # 8 token chunks
NOC = C3 // 128  # 18 qkv output chunks


def build_nc():
    nc = bacc.Bacc("TRN2", target_bir_lowering=False, debug=True)

    x_d = nc.dram_tensor("x", [N, C], FP32, kind="ExternalInput")
    wqkv_d = nc.dram_tensor("w_qkv", [C3, C], FP32, kind="ExternalInput")
    wproj_d = nc.dram_tensor("w_proj", [C, C], FP32, kind="ExternalInput")
    bproj_d = nc.dram_tensor("b_proj", [1, C], FP32, kind="ExternalInput")
    out_d = nc.dram_tensor("out", [N, C], FP32, kind="ExternalOutput")

    # DRAM views: row r = chunk*128 + partition
    x_v = x_d[:].rearrange("(c p) f -> p c f", p=128)  # [128, 8, 768]
    wqkv_v = wqkv_d[:].rearrange("(c p) f -> p c f", p=128)  # [128, 18, 768]
    wproj_v = wproj_d[:].rearrange("(c p) f -> p c f", p=128)  # [128, 6, 768]
    out_v = out_d[:].rearrange("(c p) f -> p c f", p=128)  # [128, 8, 768]

    with tile.TileContext(nc) as tc:
        with (
            tc.tile_pool(name="persist", bufs=1) as pp,
            tc.tile_pool(name="stage", bufs=3) as sp,
            tc.tile_pool(name="wstage", bufs=4) as wp,
            tc.tile_pool(name="qk", bufs=3) as qkp,
            tc.tile_pool(name="at", bufs=2) as atp,
            tc.tile_pool(name="outstage", bufs=2) as op,
        ):
            # ---- persistent SBUF ----
            xT_hi = pp.tile([128, NK, N], FP16, tag="xT_hi")  # [c%128, c//128, n]
            xT_lo = pp.tile([128, NK, N], FP16, tag="xT_lo")
            wsT = pp.tile([128, NK, C3], FP16, tag="wsT")  # sign(wqkv).T
            w2T = pp.tile([128, NK, C], FP16, tag="w2T")  # sign(wproj).T
            v_nat = pp.tile([128, NM, C], FP8, tag="v_nat")  # v, ±0.5, [m%128, m//128, hd]
            ooT = pp.tile([128, NK, N], FP16, tag="ooT")  # attn out transposed
            sc2_row = pp.tile([1, C], FP32, tag="sc2_row")  # mean|wproj| row
            sc2_rep = pp.tile([128, C], FP32, tag="sc2_rep")
            bias_row = pp.tile([1, C], FP32, tag="bias_row")
            bias_rep = pp.tile([128, C], FP32, tag="bias_rep")
            ident = pp.tile([128, 128], FP32, tag="ident")

            sigb = pp.tile([128, 1], FP32, tag="sigb")
            nc.gpsimd.memset(sigb[:], -32.0)
            make_identity(nc, ident[:])
            nc.sync.dma_start(bias_row[:], bproj_d[:])

            prep_psum = tc.tile_pool(name="ps_misc", bufs=1, space="PSUM")
            ps_misc = prep_psum.__enter__()
            vp_psum = tc.tile_pool(name="ps_v", bufs=2, space="PSUM")
            ps_v = vp_psum.__enter__()

            # ---- x: load, hi/lo fp16 split, transpose ----
            for cc in range(NM):
                xs = sp.tile([128, C], FP32, tag="x_stage")
                nc.sync.dma_start(xs[:], x_v[:, cc, :])
                hi = sp.tile([128, C], FP16, tag="hi")
                lo = sp.tile([128, C], FP16, tag="lo")
                nc.scalar.activation(hi[:], xs[:], AF.Copy)
                nc.vector.tensor_tensor(lo[:], xs[:], hi[:], ALU.subtract)
                nc.sync.dma_start_transpose(xT_hi[:, :, cc * 128 : (cc + 1) * 128], hi[:])
                nc.sync.dma_start_transpose(xT_lo[:, :, cc * 128 : (cc + 1) * 128], lo[:])

            # ---- w_proj: sign, |.| row-means, transpose ----
            sc2_ps = ps_misc.tile([1, C], FP32, tag="sc2_ps")
            for cc in range(NK):
                w2s = sp.tile([128, C], FP32, tag="w2_stage")
                nc.sync.dma_start(w2s[:], wproj_v[:, cc, :])
                w2sg = sp.tile([128, C], FP16, tag="w2_sign")
                nc.scalar.activation(w2sg[:], w2s[:], AF.Sign)
                w2abs = sp.tile([128, C], FP16, tag="w2_abs")
                sc2_col = sp.tile([128, 1], FP32, tag="sc2_col")
                nc.scalar.activation(w2abs[:], w2s[:], AF.Abs, accum_out=sc2_col[:])
                nc.tensor.transpose(
                    sc2_ps[0:1, cc * 128 : (cc + 1) * 128], sc2_col[:], ident[:]
                )
                nc.sync.dma_start_transpose(w2T[:, :, cc * 128 : (cc + 1) * 128], w2sg[:])
            nc.vector.tensor_scalar(sc2_row[:], sc2_ps[:], 1.0 / C, None, ALU.mult)
            nc.gpsimd.partition_broadcast(sc2_rep[:], sc2_row[:])
            nc.gpsimd.partition_broadcast(bias_rep[:], bias_row[:])

            # ---- w_qkv: load, sign, transpose (v chunks first) ----
            oc_order = list(range(12, 18)) + [x for p in zip(range(0, 6), range(6, 12)) for x in p]
            for oc in oc_order:
                ws = wp.tile([128, C], FP32, tag="w_stage")
                nc.sync.dma_start(ws[:], wqkv_v[:, oc, :])
                wsg = wp.tile([128, C], FP16, tag="w_sign")
                nc.scalar.activation(wsg[:], ws[:], AF.Sign)
                nc.sync.dma_start_transpose(wsT[:, :, oc * 128 : (oc + 1) * 128], wsg[:])

            # ---- v part of qkv: natural layout [m, o_v], binarize to ±0.5 ----
            for m in range(NM):
                for half in range(2):
                    vp = ps_v.tile([128, 384], FP32, tag="v_ps")
                    for k in range(NK):
                        for si, src in enumerate((xT_hi, xT_lo)):
                            nc.tensor.matmul(
                                vp[:],
                                lhsT=src[:, k, m * 128 : (m + 1) * 128],
                                rhs=wsT[:, k, 1536 + half * 384 : 1536 + (half + 1) * 384],
                                start=(k == 0 and si == 0),
                                stop=(k == NK - 1 and si == 1),
                            )
                    nc.vector.tensor_scalar(
                        v_nat[:, m, half * 384 : (half + 1) * 384],
                        vp[:],
                        0.0,
                        0.5,
                        ALU.is_ge,
                        ALU.subtract,
                    )

            vp_psum.__exit__(None, None, None)
            prep_psum.__exit__(None, None, None)

            # ---- per head-pair: q/k chunks, scores, binarize, A@V ----
            hp_psum_cms = [
                tc.tile_pool(name="ps_qk", bufs=2, space="PSUM"),
                tc.tile_pool(name="ps_s", bufs=2, space="PSUM"),
                tc.tile_pool(name="ps_oo", bufs=2, space="PSUM"),
            ]
            ps_qk, ps_s, ps_oo = [cm.__enter__() for cm in hp_psum_cms]
            for hp in range(6):
                qkT = {}
                for role, oc in (("q", hp), ("k", 6 + hp)):
                    t = qkp.tile([128, N], FP8, tag="qkT", name=f"qkT_{role}{hp}")
                    qkT[role] = t
                    for ncol in range(2):
                        qp = ps_qk.tile([128, 512], FP32, tag="qk_ps")
                        for k in range(NK):
                            for si, src in enumerate((xT_hi, xT_lo)):
                                nc.tensor.matmul(
                                    qp[:],
                                    lhsT=wsT[:, k, oc * 128 : (oc + 1) * 128],
                                    rhs=src[:, k, ncol * 512 : (ncol + 1) * 512],
                                    start=(k == 0 and si == 0),
                                    stop=(k == NK - 1 and si == 1),
                                )
                        nc.scalar.activation(
                            t[:, ncol * 512 : (ncol + 1) * 512], qp[:], AF.Sign
                        )

                at = {}
                for h01 in range(2):
                    at[h01] = atp.tile([128, NM, N], FP8, tag="at", name=f"at{hp}_{h01}")
                for m in range(NM):
                    for h01 in range(2):
                        ph = 64 * h01
                        for ncol in range(2):
                            sp_ps = ps_s.tile([128, 512], FP32, tag="s_ps")
                            nc.tensor.matmul(
                                sp_ps[:],
                                lhsT=qkT["k"][ph : ph + 64, m * 128 : (m + 1) * 128],
                                rhs=qkT["q"][ph : ph + 64, ncol * 512 : (ncol + 1) * 512],
                                tile_position=(ph, 0),
                            )
                            dst = at[h01][:, m, ncol * 512 : (ncol + 1) * 512]
                            if ncol == 0:
                                nc.scalar.activation(
                                    dst, sp_ps[:], AF.Sigmoid, bias=sigb[:], scale=32.0
                                )
                            else:
                                nc.vector.tensor_scalar(
                                    dst, sp_ps[:], 0.0, None, ALU.is_gt
                                )

                for h01 in range(2):
                    h = 2 * hp + h01
                    oo_ps = ps_oo.tile([64, N], FP32, tag="oo_ps", name=f"oo_ps{hp}_{h01}")
                    for ncol in range(2):
                        for j in range(4):
                            nc.tensor.matmul(
                                oo_ps[:, ncol * 512 : (ncol + 1) * 512],
                                lhsT=v_nat[:, 2 * j : 2 * j + 2, h * 64 : (h + 1) * 64],
                                rhs=at[h01][:, 2 * j : 2 * j + 2, ncol * 512 : (ncol + 1) * 512],
                                perf_mode=DR,
                                start=(j == 0),
                                stop=(j == 3),
                            )
                    # v was ±0.5 -> scale by 2 to recover exact integer attention out
                    if h01 == 0:
                        nc.scalar.activation(ooT[0:64, hp, :], oo_ps[:], AF.Copy, scale=2.0)
                    else:
                        # odd head's lanes must land on partitions 64-127: evac to a
                        # temp then partition-shift with a small SBUF->SBUF DMA
                        oo_tmp = op.tile([64, N], FP16, tag="oo_tmp", name=f"oo_tmp{hp}")
                        nc.vector.tensor_scalar(oo_tmp[:], oo_ps[:], 2.0, None, ALU.mult)
                        nc.sync.dma_start(ooT[64:128, hp, :], oo_tmp[:])

            for cm in reversed(hp_psum_cms):
                cm.__exit__(None, None, None)

        # ---- projection ----
        with (
            tc.tile_pool(name="proj_out", bufs=3) as pop,
            tc.tile_pool(name="ps_proj", bufs=2, space="PSUM") as ps_p,
        ):
            for m in range(NM):
                ot = pop.tile([128, C], FP32, tag="out_stage")
                for n0, nw in ((0, 512), (512, 256)):
                    pps = ps_p.tile([128, nw], FP32, tag=f"p_ps{n0}")
                    for k in range(NK):
                        nc.tensor.matmul(
                            pps[:],
                            lhsT=ooT[:, k, m * 128 : (m + 1) * 128],
                            rhs=w2T[:, k, n0 : n0 + nw],
                            start=(k == 0),
                            stop=(k == NK - 1),
                        )
                    nc.vector.scalar_tensor_tensor(
                        ot[:, n0 : n0 + nw],
                        pps[:],
                        1.0,
                        sc2_rep[:, n0 : n0 + nw],
                        ALU.bypass,
                        ALU.mult,
                    )
                nc.vector.tensor_tensor(ot[:], ot[:], bias_rep[:], ALU.add)
                nc.sync.dma_start(out_v[:, m, :], ot[:])

    nc.compile()
    return nc


_CACHE = {}


def _get_runner():
    if "runner" in _CACHE:
        return _CACHE["runner"]
    from concourse.bass_utils import run_bass_kernel_spmd

    nc = build_nc()

    def run(in_maps):
        return run_bass_kernel_spmd(nc, in_maps, core_ids=list(range(8)))

    _CACHE["runner"] = run
    return run


def kernel(x, w_qkv, w_proj, b_proj):
    x = np.asarray(x, np.float32)
    run = _get_runner()
    in_maps = [
        {
            "x": np.ascontiguousarray(x[b]),
            "w_qkv": np.asarray(w_qkv, np.float32),
            "w_proj": np.asarray(w_proj, np.float32),
            "b_proj": np.asarray(b_proj, np.float32).reshape(1, C),
        }
        for b in range(B)
    ]
    res = run(in_maps)
    return np.stack([res.results[b]["out"] for b in range(B)], axis=0)


# revision 11
# speedup vs baseline: 28.3162x; 28.3162x over previous
"""BiAttention (binary attention transformer block) Trainium2 kernel.

Forward-pass reduction of the reference:
  - softmax cancels:  stop_gradient(binq - soft) + soft == binq  (forward)
  - sign() is invariant to the positive per-row qkv weight scale
So per batch element (one per NeuronCore, 8 cores data-parallel):
  bq,bk,bv = sign(x @ sign(Wqkv).T)   split into heads
  A        = (bq @ bk.T > 0)          in {0,1}
  oo       = A @ bv                   exact small integers
  out      = (oo @ sign(Wproj).T) * mean(|Wproj|,axis=1) + b_proj
"""

import numpy as np

import concourse.bacc as bacc
import concourse.bass as bass
import concourse.mybir as mybir
import concourse.tile as tile
from concourse.masks import make_identity

FP32 = mybir.dt.float32
FP16 = mybir.dt.float16
FP8 = mybir.dt.float8e4
AF = mybir.ActivationFunctionType
ALU = mybir.AluOpType
DR = mybir.MatmulPerfMode.DoubleRow

B, N, C = 8, 1024, 768
H, D = 12, 64
C3 = 3 * C  # 2304
NK = C // 128  # 6 contraction chunks
NM = N // 128  # 8 token chunks
NOC = C3 // 128  # 18 qkv output chunks


def build_nc():
    nc = bacc.Bacc("TRN2", target_bir_lowering=False, debug=True)

    x_d = nc.dram_tensor("x", [N, C], FP32, kind="ExternalInput")
    wqkv_d = nc.dram_tensor("w_qkv", [C3, C], FP32, kind="ExternalInput")
    wproj_d = nc.dram_tensor("w_proj", [C, C], FP32, kind="ExternalInput")
    bproj_d = nc.dram_tensor("b_proj", [1, C], FP32, kind="ExternalInput")
    out_d = nc.dram_tensor("out", [N, C], FP32, kind="ExternalOutput")

    # DRAM views: row r = chunk*128 + partition
    x_v = x_d[:].rearrange("(c p) f -> p c f", p=128)  # [128, 8, 768]
    wqkv_v = wqkv_d[:].rearrange("(c p) f -> p c f", p=128)  # [128, 18, 768]
    wproj_v = wproj_d[:].rearrange("(c p) f -> p c f", p=128)  # [128, 6, 768]
    out_v = out_d[:].rearrange("(c p) f -> p c f", p=128)  # [128, 8, 768]

    with tile.TileContext(nc) as tc:
        with (
            tc.tile_pool(name="persist", bufs=1) as pp,
            tc.tile_pool(name="stage", bufs=3) as sp,
            tc.tile_pool(name="wstage", bufs=4) as wp,
            tc.tile_pool(name="qk", bufs=3) as qkp,
            tc.tile_pool(name="at", bufs=2) as atp,
            tc.tile_pool(name="outstage", bufs=2) as op,
        ):
            # ---- persistent SBUF ----
            xT_hi = pp.tile([128, NK, N], FP16, tag="xT_hi")  # [c%128, c//128, n]
            xT_lo = pp.tile([128, NK, N], FP16, tag="xT_lo")
            wsT = pp.tile([128, NK, C3], FP16, tag="wsT")  # sign(wqkv).T
            w2T = pp.tile([128, NK, C], FP16, tag="w2T")  # sign(wproj).T
            v_nat = pp.tile([128, NM, C], FP8, tag="v_nat")  # v, ±0.5, [m%128, m//128, hd]
            ooT = pp.tile([128, NK, N], FP16, tag="ooT")  # attn out transposed
            sc2_row = pp.tile([1, C], FP32, tag="sc2_row")  # mean|wproj| row
            sc2_rep = pp.tile([128, C], FP32, tag="sc2_rep")
            bias_row = pp.tile([1, C], FP32, tag="bias_row")
            bias_rep = pp.tile([128, C], FP32, tag="bias_rep")
            ident = pp.tile([128, 128], FP32, tag="ident")

            sigb = pp.tile([128, 1], FP32, tag="sigb")
            nc.gpsimd.memset(sigb[:], -32.0)
            make_identity(nc, ident[:])
            nc.sync.dma_start(bias_row[:], bproj_d[:])

            prep_psum = tc.tile_pool(name="ps_misc", bufs=1, space="PSUM")
            ps_misc = prep_psum.__enter__()
            vp_psum = tc.tile_pool(name="ps_v", bufs=2, space="PSUM")
            ps_v = vp_psum.__enter__()

            # ---- x: load, hi/lo fp16 split, transpose ----
            for cc in range(NM):
                xs = sp.tile([128, C], FP32, tag="x_stage")
                nc.sync.dma_start(xs[:], x_v[:, cc, :])
                hi = sp.tile([128, C], FP16, tag="hi")
                lo = sp.tile([128, C], FP16, tag="lo")
                nc.scalar.activation(hi[:], xs[:], AF.Copy)
                nc.vector.tensor_tensor(lo[:], xs[:], hi[:], ALU.subtract)
                nc.sync.dma_start_transpose(xT_hi[:, :, cc * 128 : (cc + 1) * 128], hi[:])
                nc.sync.dma_start_transpose(xT_lo[:, :, cc * 128 : (cc + 1) * 128], lo[:])

            # ---- w_proj: sign, |.| row-means, transpose ----
            sc2_ps = ps_misc.tile([1, C], FP32, tag="sc2_ps")
            for cc in range(NK):
                w2s = sp.tile([128, C], FP32, tag="w2_stage")
                nc.sync.dma_start(w2s[:], wproj_v[:, cc, :])
                w2sg = sp.tile([128, C], FP16, tag="w2_sign")
                nc.scalar.activation(w2sg[:], w2s[:], AF.Sign)
                w2abs = sp.tile([128, C], FP16, tag="w2_abs")
                sc2_col = sp.tile([128, 1], FP32, tag="sc2_col")
                nc.scalar.activation(w2abs[:], w2s[:], AF.Abs, accum_out=sc2_col[:])
                nc.tensor.transpose(
                    sc2_ps[0:1, cc * 128 : (cc + 1) * 128], sc2_col[:], ident[:]
                )
                nc.sync.dma_start_transpose(w2T[:, :, cc * 128 : (cc + 1) * 128], w2sg[:])
            nc.vector.tensor_scalar(sc2_row[:], sc2_ps[:], 1.0 / C, None, ALU.mult)
            nc.gpsimd.partition_broadcast(sc2_rep[:], sc2_row[:])
            nc.gpsimd.partition_broadcast(bias_rep[:], bias_row[:])

            # ---- w_qkv: load, sign, transpose (v chunks first) ----
            oc_order = list(range(12, 18)) + [x for p in zip(range(0, 6), range(6, 12)) for x in p]
            for oc in oc_order:
                ws = wp.tile([128, C], FP32, tag="w_stage")
                nc.sync.dma_start(ws[:], wqkv_v[:, oc, :])
                wsg = wp.tile([128, C], FP16, tag="w_sign")
                nc.scalar.activation(wsg[:], ws[:], AF.Sign)
                nc.sync.dma_start_transpose(wsT[:, :, oc * 128 : (oc + 1) * 128], wsg[:])

            # ---- v part of qkv: natural layout [m, o_v], binarize to ±0.5 ----
            for m in range(NM):
                for half in range(2):
                    vp = ps_v.tile([128, 384], FP32, tag="v_ps")
                    for k in range(NK):
                        for si, src in enumerate((xT_hi, xT_lo)):
                            nc.tensor.matmul(
                                vp[:],
                                lhsT=src[:, k, m * 128 : (m + 1) * 128],
                                rhs=wsT[:, k, 1536 + half * 384 : 1536 + (half + 1) * 384],
                                start=(k == 0 and si == 0),
                                stop=(k == NK - 1 and si == 1),
                            )
                    nc.vector.tensor_scalar(
                        v_nat[:, m, half * 384 : (half + 1) * 384],
                        vp[:],
                        0.0,
                        0.5,
                        ALU.is_ge,
                        ALU.subtract,
                    )

            vp_psum.__exit__(None, None, None)
            prep_psum.__exit__(None, None, None)

            # ---- per head-pair: q/k chunks, scores, binarize, A@V ----
            hp_psum_cms = [
                tc.tile_pool(name="ps_qk", bufs=2, space="PSUM"),
                tc.tile_pool(name="ps_s", bufs=2, space="PSUM"),
                tc.tile_pool(name="ps_oo", bufs=2, space="PSUM"),
            ]
            ps_qk, ps_s, ps_oo = [cm.__enter__() for cm in hp_psum_cms]
            for hp in range(6):
                qkT = {}
                for role, oc in (("q", hp), ("k", 6 + hp)):
                    t = qkp.tile([128, N], FP8, tag="qkT", name=f"qkT_{role}{hp}")
                    qkT[role] = t
                    for ncol in range(2):
                        qp = ps_qk.tile([128, 512], FP32, tag="qk_ps")
                        for k in range(NK):
                            for si, src in enumerate((xT_hi, xT_lo)):
                                nc.tensor.matmul(
                                    qp[:],
                                    lhsT=wsT[:, k, oc * 128 : (oc + 1) * 128],
                                    rhs=src[:, k, ncol * 512 : (ncol + 1) * 512],
                                    start=(k == 0 and si == 0),
                                    stop=(k == NK - 1 and si == 1),
                                )
                        nc.scalar.activation(
                            t[:, ncol * 512 : (ncol + 1) * 512], qp[:], AF.Sign
                        )

                at = {}
                for h01 in range(2):
                    at[h01] = atp.tile([128, NM, N], FP8, tag="at", name=f"at{hp}_{h01}")
                for m in range(NM):
                    for h01 in range(2):
                        ph = 64 * h01
                        for ncol in range(2):
                            sp_ps = ps_s.tile([128, 512], FP32, tag="s_ps")
                            nc.tensor.matmul(
                                sp_ps[:],
                                lhsT=qkT["k"][ph : ph + 64, m * 128 : (m + 1) * 128],
                                rhs=qkT["q"][ph : ph + 64, ncol * 512 : (ncol + 1) * 512],
                                tile_position=(ph, 0),
                            )
                            dst = at[h01][:, m, ncol * 512 : (ncol + 1) * 512]
                            if ncol == 0:
                                nc.scalar.activation(
                                    dst, sp_ps[:], AF.Sigmoid, bias=sigb[:], scale=32.0
                                )
                            else:
                                nc.vector.tensor_scalar(
                                    dst, sp_ps[:], 0.0, None, ALU.is_gt
                                )

                for h01 in range(2):
                    h = 2 * hp + h01
                    oo_ps = ps_oo.tile([64, N], FP32, tag="oo_ps", name=f"oo_ps{hp}_{h01}")
                    for ncol in range(2):
                        for j in range(4):
                            nc.tensor.matmul(
                                oo_ps[:, ncol * 512 : (ncol + 1) * 512],
                                lhsT=v_nat[:, 2 * j : 2 * j + 2, h * 64 : (h + 1) * 64],
                                rhs=at[h01][:, 2 * j : 2 * j + 2, ncol * 512 : (ncol + 1) * 512],
                                perf_mode=DR,
                                start=(j == 0),
                                stop=(j == 3),
                            )
                    # v was ±0.5 -> scale by 2 to recover exact integer attention out
                    if h01 == 0:
                        nc.scalar.activation(ooT[0:64, hp, :], oo_ps[:], AF.Copy, scale=2.0)
                    else:
                        # odd head's lanes must land on partitions 64-127: evac to a
                        # temp then partition-shift with a small SBUF->SBUF DMA
                        oo_tmp = op.tile([64, N], FP16, tag="oo_tmp", name=f"oo_tmp{hp}")
                        nc.vector.tensor_scalar(oo_tmp[:], oo_ps[:], 2.0, None, ALU.mult)
                        nc.sync.dma_start(ooT[64:128, hp, :], oo_tmp[:])

            for cm in reversed(hp_psum_cms):
                cm.__exit__(None, None, None)

        # ---- projection ----
        with (
            tc.tile_pool(name="proj_out", bufs=3) as pop,
            tc.tile_pool(name="ps_proj", bufs=2, space="PSUM") as ps_p,
        ):
            for m in range(NM):
                ot = pop.tile([128, C], FP32, tag="out_stage")
                for n0, nw in ((0, 512), (512, 256)):
                    pps = ps_p.tile([128, nw], FP32, tag=f"p_ps{n0}")
                    for k in range(NK):
                        nc.tensor.matmul(
                            pps[:],
                            lhsT=ooT[:, k, m * 128 : (m + 1) * 128],
                            rhs=w2T[:, k, n0 : n0 + nw],
                            start=(k == 0),
                            stop=(k == NK - 1),
                        )
                    nc.vector.scalar_tensor_tensor(
                        ot[:, n0 : n0 + nw],
                        pps[:],
                        1.0,
                        sc2_rep[:, n0 : n0 + nw],
                        ALU.bypass,
                        ALU.mult,
                    )
                nc.vector.tensor_tensor(ot[:], ot[:], bias_rep[:], ALU.add)
                nc.sync.dma_start(out_v[:, m, :], ot[:])

    nc.compile()
    return nc


_CACHE = {}


def _get_exec():
    """Build (once) and cache a jitted SPMD executable for the 8-core kernel."""
    if "exec" in _CACHE:
        return _CACHE["exec"]
    import jax
    import concourse.mybir as _mybir
    from jax.sharding import Mesh, PartitionSpec
    from jax.experimental.shard_map import shard_map
    from concourse.bass2jax import _bass_exec_p, install_neuronx_cc_hook

    nc = build_nc()
    install_neuronx_cc_hook()

    in_names, out_names, out_avals = [], [], []
    for alloc in nc.m.functions[0].allocations:
        if not isinstance(alloc, _mybir.MemoryLocationSet):
            continue
        name = alloc.memorylocations[0].name
        if alloc.kind == "ExternalInput":
            if name not in ("dbg_addr", "partition_id"):
                in_names.append(name)
        elif alloc.kind == "ExternalOutput":
            out_names.append(name)
            out_avals.append(
                jax.core.ShapedArray(tuple(alloc.tensor_shape), _mybir.dt.np(alloc.dtype))
            )
    if nc.dbg_addr is not None:
        in_names.append(nc.dbg_addr.name)
    n_params = len(in_names)
    n_outs = len(out_names)
    partition_name = nc.partition_id_tensor.name if nc.partition_id_tensor else None
    all_in_names = tuple(
        in_names + out_names + ([partition_name] if partition_name else [])
    )
    donate = tuple(range(n_params, n_params + n_outs))

    def _body(*args):
        operands = list(args)
        if partition_name is not None:
            from concourse.bass2jax import partition_id_tensor

            operands.append(partition_id_tensor())
        outs = _bass_exec_p.bind(
            *operands,
            out_avals=tuple(out_avals),
            in_names=all_in_names,
            out_names=tuple(out_names),
            lowering_input_output_aliases=(),
            sim_require_finite=True,
            sim_require_nnan=True,
            nc=nc,
        )
        return tuple(outs)

    devices = jax.devices()[:B]
    mesh = Mesh(np.array(devices), ("core",))
    in_specs = (PartitionSpec("core"),) * (n_params + n_outs)
    out_specs = (PartitionSpec("core"),) * n_outs
    sharded = jax.jit(
        shard_map(_body, mesh=mesh, in_specs=in_specs, out_specs=out_specs, check_rep=False),
        donate_argnums=donate,
        keep_unused=True,
    )
    _CACHE["exec"] = (sharded, in_names, out_names, out_avals, mesh)
    return _CACHE["exec"]


def _concat_inputs(x, w_qkv, w_proj, b_proj):
    """Per-core inputs concatenated along axis 0 (shard_map convention)."""
    x = np.asarray(x, np.float32)
    w_qkv = np.asarray(w_qkv, np.float32)
    w_proj = np.asarray(w_proj, np.float32)
    b_proj = np.asarray(b_proj, np.float32).reshape(1, C)
    per_core = {
        "x": [np.ascontiguousarray(x[b]) for b in range(B)],
        "w_qkv": [w_qkv] * B,
        "w_proj": [w_proj] * B,
        "b_proj": [b_proj] * B,
        "dbg_addr": [np.zeros((1, 2), np.uint32)] * B,
    }
    return per_core


def _zero_outs(out_names, out_avals):
    return [
        np.zeros((B * a.shape[0], *a.shape[1:]), a.dtype) for a in out_avals
    ]


def kernel(x, w_qkv, w_proj, b_proj):
    sharded, in_names, out_names, out_avals, mesh = _get_exec()
    per_core = _concat_inputs(x, w_qkv, w_proj, b_proj)
    concat_in = [np.concatenate(per_core[name], axis=0) for name in in_names]
    out_arrs = sharded(*concat_in, *_zero_outs(out_names, out_avals))
    i = out_names.index("out")
    a = out_avals[i]
    return np.asarray(out_arrs[i]).reshape(B, *a.shape)


# revision 22
# speedup vs baseline: 12139.4018x; 428.7095x over previous
"""BiAttention (binary attention transformer block) Trainium2 kernel.

Forward-pass reduction of the reference:
  - softmax cancels:  stop_gradient(binq - soft) + soft == binq  (forward)
  - sign() is invariant to the positive per-row qkv weight scale
So per batch element (one per NeuronCore, 8 cores data-parallel):
  bq,bk,bv = sign(x @ sign(Wqkv).T)   split into heads
  A        = (bq @ bk.T > 0)          in {0,1}
  oo       = A @ bv                   exact small integers
  out      = (oo @ sign(Wproj).T) * mean(|Wproj|,axis=1) + b_proj
"""

import numpy as np

import concourse.bacc as bacc
import concourse.bass as bass
import concourse.mybir as mybir
import concourse.tile as tile
from concourse.masks import make_identity

FP32 = mybir.dt.float32
FP16 = mybir.dt.float16
FP8 = mybir.dt.float8e4
AF = mybir.ActivationFunctionType
ALU = mybir.AluOpType
DR = mybir.MatmulPerfMode.DoubleRow

B, N, C = 8, 1024, 768
H, D = 12, 64
C3 = 3 * C  # 2304
NK = C // 128  # 6 contraction chunks
NM = N // 128  # 8 token chunks
NOC = C3 // 128  # 18 qkv output chunks


QKV_MODE = "hilo"  # "hilo" (fp16 two-pass, exact) or "f32r" (single-pass float32r)
WT_MODE = "pe"  # "pe" (transpose f32 on PE, sign on evac) or "xbar" (sign then DMA-transpose)


def build_nc(repeat=1):
    nc = bacc.Bacc("TRN2", target_bir_lowering=False, debug=True)

    x_d = nc.dram_tensor("x", [N, C], FP32, kind="ExternalInput")
    wqkv_d = nc.dram_tensor("w_qkv", [C3, C], FP32, kind="ExternalInput")
    wproj_d = nc.dram_tensor("w_proj", [C, C], FP32, kind="ExternalInput")
    bproj_d = nc.dram_tensor("b_proj", [1, C], FP32, kind="ExternalInput")
    out_d = nc.dram_tensor("out", [N, C], FP32, kind="ExternalOutput")

    # DRAM views: row r = chunk*128 + partition
    x_v = x_d[:].rearrange("(c p) f -> p c f", p=128)  # [128, 8, 768]
    wqkv_v = wqkv_d[:].rearrange("(c p) f -> p c f", p=128)  # [128, 18, 768]
    wproj_v = wproj_d[:].rearrange("(c p) f -> p c f", p=128)  # [128, 6, 768]
    out_v = out_d[:].rearrange("(c p) f -> p c f", p=128)  # [128, 8, 768]

    with tile.TileContext(nc) as tc:
        for _rep in range(repeat):
            _emit_body(nc, tc, _rep, x_v, wqkv_v, wproj_v, bproj_d, out_v)

    nc.compile()
    return nc


def _emit_body(nc, tc, rep, x_v, wqkv_v, wproj_v, bproj_d, out_v):
    _p = f"r{rep}_"
    if True:
        with (
            tc.tile_pool(name=_p + "persist", bufs=1) as pp,
            tc.tile_pool(name=_p + "stage", bufs=3) as sp,
            tc.tile_pool(name=_p + "wstage", bufs=4) as wp,
            tc.tile_pool(name=_p + "qk", bufs=3) as qkp,
            tc.tile_pool(name=_p + "at", bufs=2) as atp,
            tc.tile_pool(name=_p + "outstage", bufs=2) as op,
        ):
            # ---- persistent SBUF ----
            FPR = mybir.dt.float32r
            if QKV_MODE == "hilo":
                xT_hi = pp.tile([128, NK, N], FP16, tag="xT_hi")  # [c%128, c//128, n]
                xT_lo = pp.tile([128, NK, N], FP16, tag="xT_lo")
                wsT = pp.tile([128, NK, C3], FP16, tag="wsT")  # sign(wqkv).T
                qkv_srcs = (xT_hi, xT_lo)
            else:
                xT_r = pp.tile([128, NK, N], FPR, tag="xT_r")
                wsT = pp.tile([128, NK, C3], FPR, tag="wsT")
                qkv_srcs = (xT_r,)
            w2T = pp.tile([128, NK, C], FP16, tag="w2T")  # sign(wproj).T
            v_nat = pp.tile([128, NM, C], FP8, tag="v_nat")  # v, ±0.5, [m%128, m//128, hd]
            ooT = pp.tile([128, NK, N], FP16, tag="ooT")  # attn out transposed
            sc2_row = pp.tile([1, C], FP32, tag="sc2_row")  # mean|wproj| row
            sc2_rep = pp.tile([128, C], FP32, tag="sc2_rep")
            bias_row = pp.tile([1, C], FP32, tag="bias_row")
            bias_rep = pp.tile([128, C], FP32, tag="bias_rep")
            ident = pp.tile([128, 128], FP32, tag="ident")

            sigb = pp.tile([128, 1], FP32, tag="sigb")
            nc.gpsimd.memset(sigb[:], -32.0)
            make_identity(nc, ident[:])
            nc.sync.dma_start(bias_row[:], bproj_d[:])

            # ---- prep phase: loads + PE transposes (PE is otherwise idle) ----
            tr_pool_cm = tc.tile_pool(name=_p + "ps_tr", bufs=3, space="PSUM")
            ps_tr = tr_pool_cm.__enter__()
            misc_cm = [None]

            # x: load [n,c] chunks, transpose on PE, split into fp16 hi/lo
            for cc in range(NM):
                xs = sp.tile([128, C], FP32, tag="x_stage")
                nc.sync.dma_start(xs[:], x_v[:, cc, :])
                xtp = ps_tr.tile([128, C], FP32, tag="tr_ps", name=f"xtr{cc}")
                for k in range(NK):
                    nc.tensor.transpose(
                        xtp[:, k * 128 : (k + 1) * 128],
                        xs[:, k * 128 : (k + 1) * 128],
                        ident[:],
                    )
                if QKV_MODE == "hilo":
                    dst_hi = xT_hi[:, :, cc * 128 : (cc + 1) * 128]
                    dst_lo = xT_lo[:, :, cc * 128 : (cc + 1) * 128]
                    nc.scalar.activation(dst_hi, xtp[:], AF.Copy)
                    nc.vector.tensor_tensor(dst_lo, xtp[:], dst_hi, ALU.subtract)
                else:
                    nc.scalar.activation(
                        xT_r[:, :, cc * 128 : (cc + 1) * 128], xtp[:], AF.Copy
                    )

            # w_proj: sign+transpose; |.| row-means via accum  (emitted after
            # the w_qkv/v-part phase: its results are only needed by proj)
            def emit_w2_prep():
              misc_cm[0] = tc.tile_pool(name=_p + "ps_misc", bufs=1, space="PSUM")
              ps_misc = misc_cm[0].__enter__()
              sc2_ps = ps_misc.tile([1, C], FP32, tag="sc2_ps")
              for cc in range(NK):
                w2s = sp.tile([128, C], FP32, tag="w2_stage", name=f"w2s{cc}")
                nc.sync.dma_start(w2s[:], wproj_v[:, cc, :])
                w2abs = sp.tile([128, C], FP16, tag="w2_abs", name=f"w2abs{cc}")
                sc2_col = sp.tile([128, 1], FP32, tag="sc2_col", name=f"sc2c{cc}")
                nc.scalar.activation(w2abs[:], w2s[:], AF.Abs, accum_out=sc2_col[:])
                nc.tensor.transpose(
                    sc2_ps[0:1, cc * 128 : (cc + 1) * 128], sc2_col[:], ident[:]
                )
                if WT_MODE == "pe":
                    w2tp = ps_tr.tile([128, C], FP32, tag="tr_ps", name=f"w2tr{cc}")
                    for k in range(NK):
                        nc.tensor.transpose(
                            w2tp[:, k * 128 : (k + 1) * 128],
                            w2s[:, k * 128 : (k + 1) * 128],
                            ident[:],
                        )
                    nc.scalar.activation(
                        w2T[:, :, cc * 128 : (cc + 1) * 128], w2tp[:], AF.Sign
                    )
                else:
                    w2sg = sp.tile([128, C], FP16, tag="w2_sign", name=f"w2sg{cc}")
                    nc.scalar.activation(w2sg[:], w2s[:], AF.Sign)
                    nc.sync.dma_start_transpose(
                        w2T[:, :, cc * 128 : (cc + 1) * 128], w2sg[:]
                    )
              nc.vector.tensor_scalar(sc2_row[:], sc2_ps[:], 1.0 / C, None, ALU.mult)
              nc.gpsimd.partition_broadcast(sc2_rep[:], sc2_row[:])
              nc.gpsimd.partition_broadcast(bias_rep[:], bias_row[:])

            # w_qkv: load, transpose on PE, sign -> fp16 wsT (v chunks first)
            oc_order = list(range(12, 18)) + [
                x for pair in zip(range(0, 6), range(6, 12)) for x in pair
            ]
            vpart_emitted = False
            vp_cm = None

            def emit_v_part():
                for m in range(NM):
                    for half in range(2):
                        vp = ps_v.tile([128, 384], FP32, tag="v_ps", name=f"vps{m}_{half}")
                        ns = len(qkv_srcs)
                        for k in range(NK):
                            for si, src in enumerate(qkv_srcs):
                                nc.tensor.matmul(
                                    vp[:],
                                    lhsT=src[:, k, m * 128 : (m + 1) * 128],
                                    rhs=wsT[:, k, 1536 + half * 384 : 1536 + (half + 1) * 384],
                                    start=(k == 0 and si == 0),
                                    stop=(k == NK - 1 and si == ns - 1),
                                )
                        nc.vector.tensor_scalar(
                            v_nat[:, m, half * 384 : (half + 1) * 384],
                            vp[:],
                            0.0,
                            0.5,
                            ALU.is_ge,
                            ALU.subtract,
                        )

            for i, oc in enumerate(oc_order):
                ws = wp.tile([128, C], FP32, tag="w_stage")
                nc.sync.dma_start(ws[:], wqkv_v[:, oc, :])
                if WT_MODE == "pe" or QKV_MODE == "f32r":
                    wtp = ps_tr.tile([128, C], FP32, tag="tr_ps", name=f"wtr{oc}")
                    for k in range(NK):
                        nc.tensor.transpose(
                            wtp[:, k * 128 : (k + 1) * 128],
                            ws[:, k * 128 : (k + 1) * 128],
                            ident[:],
                        )
                    nc.scalar.activation(
                        wsT[:, :, oc * 128 : (oc + 1) * 128], wtp[:], AF.Sign
                    )
                else:
                    wsg = wp.tile([128, C], FP16, tag="w_sign")
                    nc.scalar.activation(wsg[:], ws[:], AF.Sign)
                    nc.sync.dma_start_transpose(
                        wsT[:, :, oc * 128 : (oc + 1) * 128], wsg[:]
                    )
                if i == 5:
                    # v-slice of wsT complete: emit the v-part matmuls now so
                    # they overlap the remaining q/k chunk loads
                    vp_cm = tc.tile_pool(name=_p + "ps_v", bufs=2, space="PSUM")
                    ps_v = vp_cm.__enter__()
                    emit_v_part()
                    vpart_emitted = True
            assert vpart_emitted
            vp_cm.__exit__(None, None, None)
            emit_w2_prep()
            misc_cm[0].__exit__(None, None, None)
            tr_pool_cm.__exit__(None, None, None)

            # ---- per head-pair: q/k chunks, scores, binarize, A@V ----
            hp_psum_cms = [
                tc.tile_pool(name=_p + "ps_qk", bufs=2, space="PSUM"),
                tc.tile_pool(name=_p + "ps_s", bufs=2, space="PSUM"),
                tc.tile_pool(name=_p + "ps_oo", bufs=1, space="PSUM"),
            ]
            ps_qk, ps_s, ps_oo = [cm.__enter__() for cm in hp_psum_cms]
            bin_idx = 0
            qkTs = {}

            def emit_qk(hp):
                qkT = {}
                for role, oc in (("q", hp), ("k", 6 + hp)):
                    t = qkp.tile([128, N], FP8, tag="qkT", name=f"qkT_{role}{hp}")
                    qkT[role] = t
                    for ncol in range(2):
                        qp = ps_qk.tile([128, 512], FP32, tag="qk_ps")
                        ns = len(qkv_srcs)
                        for k in range(NK):
                            for si, src in enumerate(qkv_srcs):
                                nc.tensor.matmul(
                                    qp[:],
                                    lhsT=wsT[:, k, oc * 128 : (oc + 1) * 128],
                                    rhs=src[:, k, ncol * 512 : (ncol + 1) * 512],
                                    start=(k == 0 and si == 0),
                                    stop=(k == NK - 1 and si == ns - 1),
                                )
                        nc.scalar.activation(
                            t[:, ncol * 512 : (ncol + 1) * 512], qp[:], AF.Sign
                        )
                qkTs[hp] = qkT

            emit_qk(0)
            for hp in range(6):
                qkT = qkTs.pop(hp)
                at = {}
                for h01 in range(2):
                    at[h01] = atp.tile([128, NM, N], FP8, tag="at", name=f"at{hp}_{h01}")
                for m in range(NM):
                    for h01 in range(2):
                        ph = 64 * h01
                        sp_ps = ps_s.tile([128, N], FP32, tag="s_ps", name=f"sps{hp}_{m}_{h01}")
                        for ncol in range(2):
                            nc.tensor.matmul(
                                sp_ps[:, ncol * 512 : (ncol + 1) * 512],
                                lhsT=qkT["k"][ph : ph + 64, m * 128 : (m + 1) * 128],
                                rhs=qkT["q"][ph : ph + 64, ncol * 512 : (ncol + 1) * 512],
                                tile_position=(ph, 0),
                            )
                        dst = at[h01][:, m, :]
                        if bin_idx % 2 == 0:
                            nc.scalar.activation(
                                dst, sp_ps[:], AF.Sigmoid, bias=sigb[:], scale=32.0
                            )
                        else:
                            nc.vector.tensor_scalar(dst, sp_ps[:], 0.0, None, ALU.is_gt)
                        bin_idx += 1

                if hp + 1 < 6:
                    emit_qk(hp + 1)

                for h01 in range(2):
                    h = 2 * hp + h01
                    oo_ps = ps_oo.tile([64, N], FP32, tag="oo_ps", name=f"oo_ps{hp}_{h01}")
                    for j in range(4):
                        for ncol in range(2):
                            nc.tensor.matmul(
                                oo_ps[:, ncol * 512 : (ncol + 1) * 512],
                                lhsT=v_nat[:, 2 * j : 2 * j + 2, h * 64 : (h + 1) * 64],
                                rhs=at[h01][:, 2 * j : 2 * j + 2, ncol * 512 : (ncol + 1) * 512],
                                perf_mode=DR,
                                start=(j == 0),
                                stop=(j == 3),
                            )
                    # v was ±0.5 -> scale by 2 to recover exact integer attention out
                    if h01 == 0:
                        dsth = ooT[0:64, hp, :]
                        nc.scalar.activation(
                            dsth[:, 0:512], oo_ps[:, 0:512], AF.Copy, scale=2.0
                        )
                        nc.vector.tensor_scalar(
                            dsth[:, 512:1024], oo_ps[:, 512:1024], 2.0, None, ALU.mult
                        )
                    else:
                        # odd head's lanes must land on partitions 64-127: evac to a
                        # temp then partition-shift with a small SBUF->SBUF DMA
                        oo_tmp = op.tile([64, N], FP16, tag="oo_tmp", name=f"oo_tmp{hp}")
                        nc.scalar.activation(
                            oo_tmp[:, 0:512], oo_ps[:, 0:512], AF.Copy, scale=2.0
                        )
                        nc.vector.tensor_scalar(
                            oo_tmp[:, 512:1024], oo_ps[:, 512:1024], 2.0, None, ALU.mult
                        )
                        nc.sync.dma_start(ooT[64:128, hp, :], oo_tmp[:])
            for cm in reversed(hp_psum_cms):
                cm.__exit__(None, None, None)

        # ---- projection ----
        with (
            tc.tile_pool(name=_p + "proj_out", bufs=3) as pop,
            tc.tile_pool(name=_p + "ps_proj", bufs=2, space="PSUM") as ps_p,
        ):
            for m in range(NM):
                ot = pop.tile([128, C], FP32, tag="out_stage")
                for n0, nw in ((0, 512), (512, 256)):
                    pps = ps_p.tile([128, nw], FP32, tag=f"p_ps{n0}")
                    for k in range(NK):
                        nc.tensor.matmul(
                            pps[:],
                            lhsT=ooT[:, k, m * 128 : (m + 1) * 128],
                            rhs=w2T[:, k, n0 : n0 + nw],
                            start=(k == 0),
                            stop=(k == NK - 1),
                        )
                    nc.vector.scalar_tensor_tensor(
                        ot[:, n0 : n0 + nw],
                        pps[:],
                        1.0,
                        sc2_rep[:, n0 : n0 + nw],
                        ALU.bypass,
                        ALU.mult,
                    )
                nc.vector.tensor_tensor(ot[:], ot[:], bias_rep[:], ALU.add)
                nc.sync.dma_start(out_v[:, m, :], ot[:])


_CACHE = {}


def _get_exec():
    """Build (once) and cache a jitted SPMD executable for the 8-core kernel."""
    if "exec" in _CACHE:
        return _CACHE["exec"]
    import jax
    import concourse.mybir as _mybir
    from jax.sharding import Mesh, PartitionSpec
    from jax.experimental.shard_map import shard_map
    from concourse.bass2jax import _bass_exec_p, install_neuronx_cc_hook

    nc = build_nc()
    install_neuronx_cc_hook()

    in_names, out_names, out_avals = [], [], []
    for alloc in nc.m.functions[0].allocations:
        if not isinstance(alloc, _mybir.MemoryLocationSet):
            continue
        name = alloc.memorylocations[0].name
        if alloc.kind == "ExternalInput":
            if name not in ("dbg_addr", "partition_id"):
                in_names.append(name)
        elif alloc.kind == "ExternalOutput":
            out_names.append(name)
            out_avals.append(
                jax.core.ShapedArray(tuple(alloc.tensor_shape), _mybir.dt.np(alloc.dtype))
            )
    if nc.dbg_addr is not None:
        in_names.append(nc.dbg_addr.name)
    n_params = len(in_names)
    n_outs = len(out_names)
    partition_name = nc.partition_id_tensor.name if nc.partition_id_tensor else None
    all_in_names = tuple(
        in_names + out_names + ([partition_name] if partition_name else [])
    )
    donate = tuple(range(n_params, n_params + n_outs))

    def _body(*args):
        operands = list(args)
        if partition_name is not None:
            from concourse.bass2jax import partition_id_tensor

            operands.append(partition_id_tensor())
        outs = _bass_exec_p.bind(
            *operands,
            out_avals=tuple(out_avals),
            in_names=all_in_names,
            out_names=tuple(out_names),
            lowering_input_output_aliases=(),
            sim_require_finite=True,
            sim_require_nnan=True,
            nc=nc,
        )
        return tuple(outs)

    devices = jax.devices()[:B]
    mesh = Mesh(np.array(devices), ("core",))
    in_specs = (PartitionSpec("core"),) * (n_params + n_outs)
    out_specs = (PartitionSpec("core"),) * n_outs
    sharded = jax.jit(
        shard_map(_body, mesh=mesh, in_specs=in_specs, out_specs=out_specs, check_rep=False),
        donate_argnums=donate,
        keep_unused=True,
    )
    _CACHE["exec"] = (sharded, in_names, out_names, out_avals, mesh)
    return _CACHE["exec"]


def _concat_inputs(x, w_qkv, w_proj, b_proj):
    """Per-core inputs concatenated along axis 0 (shard_map convention)."""
    x = np.asarray(x, np.float32)
    w_qkv = np.asarray(w_qkv, np.float32)
    w_proj = np.asarray(w_proj, np.float32)
    b_proj = np.asarray(b_proj, np.float32).reshape(1, C)
    per_core = {
        "x": [np.ascontiguousarray(x[b]) for b in range(B)],
        "w_qkv": [w_qkv] * B,
        "w_proj": [w_proj] * B,
        "b_proj": [b_proj] * B,
        "dbg_addr": [np.zeros((1, 2), np.uint32)] * B,
    }
    return per_core


def _zero_outs(out_names, out_avals):
    return [
        np.zeros((B * a.shape[0], *a.shape[1:]), a.dtype) for a in out_avals
    ]


def kernel(x, w_qkv, w_proj, b_proj):
    sharded, in_names, out_names, out_avals, mesh = _get_exec()
    per_core = _concat_inputs(x, w_qkv, w_proj, b_proj)
    concat_in = [np.concatenate(per_core[name], axis=0) for name in in_names]
    out_arrs = sharded(*concat_in, *_zero_outs(out_names, out_avals))
    i = out_names.index("out")
    a = out_avals[i]
    return np.asarray(out_arrs[i]).reshape(B, *a.shape)
